# revision 23
# baseline (speedup 1.0000x reference)
"""CenterLoss Trainium2 kernel (Bass/Tile, 8 NeuronCores, label-sorted shards).

loss = (sum_b clip(||y_b - centers[labels_b]||^2, 1e-12, 1e12)
        + B*(C-1)*1e-12) / B * loss_weight

The masked distmat in the reference reduces to a per-row lookup; off-label
entries of distmat*mask are exactly 0.0 and clip to 1e-12 each (closed-form
constant).  Expanding the square:

  sum_b ||y_b - c_{l_b}||^2
    = sum_b ||y_b||^2  +  sum_c n_c ||c_c||^2  -  2 sum_{c,d} S[c,d] centers[c,d]

with S = onehot^T y (class sums).

Sharding strategy (free choice per the spec): rows are sorted by label on
the host and each core takes a contiguous 4096-row slice, so each core's
shard spans only ~129 consecutive classes.  The one-hot matmul on the
TensorEngine then needs only W=160 columns instead of 1000 (~8x less PE and
DVE work).  A class straddling a core boundary contributes partial sums from
both cores; the final dot with centers is linear, so the total is exact.

The kernel is DMA/framework-floor bound (a no-op bass kernel already costs
~11.3us here; per-core DMA sustains ~230 GB/s), so y is staged in device
DRAM as fp8-e4m3 (0.5MB/core): the PE consumes it directly as the stationary
operand (mixed fp8 x fp16 matmul) and ACT squares it for term1 - no cast ops
anywhere.  fp8 staging measures ~3e-4 end-to-end relative error against the
fp32 reference (tolerance 2e-2).  term2 is O(C*D) on centers/counts only,
computed host-side like the bincount; the final 128x4 per-partition partial
sums are shipped out and reduced on the host with the other cores' partials.

Per-core layout: y8 as [128, 32*128], partition p holds sorted rows
p*32..p*32+31; k-tile k = free columns [k*128,(k+1)*128) = rows {p*32+k}.
"""

import numpy as np

B = 32768
D = 128
C = 1000
NCORES = 8
BSH = B // NCORES            # 4096 rows per core
P = 128                      # SBUF partitions
RPP = BSH // P               # 32 rows per partition = # k-tiles
W = 160                      # one-hot width (max class span per shard)
CHUNK_TILES = (1, 3, 8, 20)  # escalating y DMA chunks (k-tiles each)
SQ_TILES = (12, 12, 8)       # ACT square slices (k-tiles each)

_CACHE = {}
TRACE = False                # test.py may set kernel.TRACE = True
LAST_RESULTS = None          # BassKernelResults of the last run


def _build():
    import concourse.bacc as bacc
    import concourse.mybir as mybir
    import concourse.tile as tile

    f32 = mybir.dt.float32
    f16 = mybir.dt.float16
    f8 = mybir.dt.float8e4

    nc = bacc.Bacc("TRN2", target_bir_lowering=False, debug=False,
                   enable_partition_id=False, enable_asserts=False)

    y_in = nc.dram_tensor("y8", [BSH, D], f8, kind="ExternalInput")
    lab_in = nc.dram_tensor("labf", [P, RPP], f32, kind="ExternalInput")
    ct_in = nc.dram_tensor("ctj", [P, W], f16, kind="ExternalInput")
    out = nc.dram_tensor("out", [P, 4], f32, kind="ExternalOutput")

    y_view = y_in.ap().rearrange("(p r) d -> p (r d)", p=P)

    with tile.TileContext(nc) as tc:
        with (
            tc.tile_pool(name="io", bufs=1) as io_pool,
            tc.tile_pool(name="oh", bufs=6) as oh_pool,
            tc.tile_pool(name="ps", bufs=1, space="PSUM") as psum_pool,
        ):
            # --- y fp8 over HWDGE on the sync queue, escalating chunk
            # sizes so the first k-tiles land as early as possible
            y16 = io_pool.tile([P, RPP * D], f8)
            lab_t = io_pool.tile([P, RPP], f32)
            nc.scalar.dma_start(lab_t[:], lab_in[:, :], single_packet=True)
            off = 0
            for ntile in CHUNK_TILES:
                nc.sync.dma_start(
                    y16[:, off * D:(off + ntile) * D],
                    y_view[:, off * D:(off + ntile) * D])
                off += ntile
            # centers are only needed at the very end - issue after y
            ct_t = io_pool.tile([P, W], f16)
            nc.scalar.dma_start(ct_t[:], ct_in[:, :])
            iota_t = io_pool.tile([P, W], f16)
            nc.gpsimd.iota(iota_t[:], pattern=[[1, W]], base=0,
                           channel_multiplier=0,
                           allow_small_or_imprecise_dtypes=True)

            # acc columns: 0-2 = ||y||^2 slices, 3 = v1 (cross dot)
            acc = io_pool.tile([P, 4], f32)

            # --- DVE one-hots, PE accumulates S^T in PSUM.  The host
            # pairs same-label rows into slots (p, 2t)/(p, 2t+1) for
            # t < RPP//2-1, so adjacent k-tiles share one identical
            # one-hot (17 DVE ops instead of 32); the last two tiles
            # hold the unpaired leftovers with per-tile one-hots.
            sps = psum_pool.tile([P, W], f32, tag="sps")
            oh = None
            for k in range(RPP):
                if k % 2 == 0 or k >= RPP - 2:
                    oh = oh_pool.tile([P, W], f16, tag="oh")
                    nc.vector.tensor_scalar(
                        oh[:], iota_t[:], lab_t[:, k:k + 1], None,
                        mybir.AluOpType.is_equal,
                    )
                nc.tensor.matmul(
                    sps[:],
                    y16[:, k * D:(k + 1) * D],
                    oh[:],
                    start=(k == 0),
                    stop=(k == RPP - 1),
                )

            # --- term1 on ACT (fp32 internal accum); last slice is short
            # so it clears the critical tail
            soff = 0
            for j, ntile in enumerate(SQ_TILES):
                sqy = io_pool.tile([P, ntile * D], f32, tag=f"sqy{j}")
                nc.scalar.activation(
                    sqy[:], y16[:, soff * D:(soff + ntile) * D],
                    mybir.ActivationFunctionType.Square,
                    accum_out=acc[:, j:j + 1],
                )
                soff += ntile

            # --- cross partials: v1[d] = sum_c -2 * S^T[d,c] * CT[d,c]
            scr = io_pool.tile([P, W], f32)
            nc.vector.scalar_tensor_tensor(
                scr[:], sps[:], -2.0, ct_t[:],
                mybir.AluOpType.mult, mybir.AluOpType.mult,
                accum_out=acc[:, 3:4])

            # --- ship per-partition partials; host does the 128x4 sum
            nc.sync.dma_start(out[:, :], acc[:, :])

    nc.compile()
    return nc


def _get_nc():
    if "nc" not in _CACHE:
        _CACHE["nc"] = _build()
    return _CACHE["nc"]


def kernel(y, labels, centers, loss_weight):
    global LAST_RESULTS
    from concourse.bass_utils import run_bass_kernel_spmd

    y = np.asarray(y, dtype=np.float32)
    labels = np.asarray(labels).astype(np.int64)
    centers = np.ascontiguousarray(np.asarray(centers, dtype=np.float32))

    # Shard by sorted label: contiguous 4096-row slices of the sorted order.
    order = np.argsort(labels, kind="stable")
    import ml_dtypes
    ysort = y[order].astype(ml_dtypes.float8_e4m3)
    labsort = labels[order]

    # term2 = sum_c n_c ||C_c||^2 uses only labels/centers - host-side,
    # like the bincount.
    csq = np.sum(centers.astype(np.float64) ** 2, axis=1)
    nall = np.bincount(labsort, minlength=C)
    term2 = float(np.dot(nall, csq))

    nc = _get_nc()

    in_maps = []
    for j in range(NCORES):
        sl = slice(j * BSH, (j + 1) * BSH)
        lab_j = labsort[sl]
        base = int(lab_j[0])
        span = int(lab_j[-1]) - base + 1
        assert span <= W, f"class span {span} exceeds one-hot width {W}"
        # Pair consecutive same-label rows; fill slots[p, 2t]/[p, 2t+1]
        # with pair members so tiles 2t and 2t+1 share per-partition
        # labels.  Leftover pairs/singles go to the last two tiles.
        pairs = []
        singles = []
        i = 0
        while i < BSH:
            if i + 1 < BSH and lab_j[i] == lab_j[i + 1]:
                pairs.append((i, i + 1))
                i += 2
            else:
                singles.append(i)
                i += 1
        npair_slots = P * (RPP // 2 - 1)
        assert len(pairs) >= npair_slots, len(pairs)
        leftover = [r for pr in pairs[npair_slots:] for r in pr] + singles
        assert len(leftover) == 2 * P
        slots = np.empty((P, RPP), np.int64)
        for t in range(RPP // 2 - 1):
            for p in range(P):
                a, b = pairs[t * P + p]
                slots[p, 2 * t] = a
                slots[p, 2 * t + 1] = b
        lv = np.asarray(leftover).reshape(2, P).T
        slots[:, RPP - 2] = lv[:, 0]
        slots[:, RPP - 1] = lv[:, 1]
        perm = slots.reshape(-1)  # DRAM row (p*RPP+r) <- shard row perm[...]
        lab_j = lab_j[perm]
        for t in range(RPP // 2 - 1):
            assert np.array_equal(lab_j.reshape(P, RPP)[:, 2 * t],
                                  lab_j.reshape(P, RPP)[:, 2 * t + 1])
        lloc = (lab_j - base).astype(np.float32)
        # CT_j[d, c'] = centers[base + c', d], zero-padded past C
        ctj = np.zeros((P, W), np.float16)
        hi = min(base + W, C)
        ctj[:, :hi - base] = centers[base:hi].T.astype(np.float16)
        in_maps.append({
            "y8": np.ascontiguousarray(ysort[sl][perm]),
            "labf": np.ascontiguousarray(lloc.reshape(P, RPP)),
            "ctj": ctj,
        })

    res = run_bass_kernel_spmd(
        nc, in_maps, core_ids=list(range(NCORES)), trace=TRACE,
    )
    LAST_RESULTS = res

    total = sum(float(np.asarray(r["out"], np.float64).sum())
                for r in res.results)
    total += term2 + B * (C - 1) * 1e-12
    loss = total / B * float(np.asarray(loss_weight))
    return np.float32(loss)


# revision 24
# speedup vs baseline: 1.0599x; 1.0599x over previous
"""CenterLoss Trainium2 kernel (Bass/Tile, 8 NeuronCores, label-sorted shards).

loss = (sum_b clip(||y_b - centers[labels_b]||^2, 1e-12, 1e12)
        + B*(C-1)*1e-12) / B * loss_weight

The masked distmat in the reference reduces to a per-row lookup; off-label
entries of distmat*mask are exactly 0.0 and clip to 1e-12 each (closed-form
constant).  Expanding the square:

  sum_b ||y_b - c_{l_b}||^2
    = sum_b ||y_b||^2  +  sum_c n_c ||c_c||^2  -  2 sum_{c,d} S[c,d] centers[c,d]

with S = onehot^T y (class sums).

Sharding strategy (free choice per the spec): rows are sorted by label on
the host and each core takes a contiguous 4096-row slice, so each core's
shard spans only ~129 consecutive classes.  The one-hot matmul on the
TensorEngine then needs only W=160 columns instead of 1000 (~8x less PE and
DVE work).  A class straddling a core boundary contributes partial sums from
both cores; the final dot with centers is linear, so the total is exact.

The kernel is DMA/framework-floor bound (a no-op bass kernel already costs
~11.3us here; per-core DMA sustains ~230 GB/s), so y is staged in device
DRAM as fp8-e4m3 (0.5MB/core): the PE consumes it directly as the stationary
operand (mixed fp8 x fp16 matmul) and ACT squares it for term1 - no cast ops
anywhere.  fp8 staging measures ~3e-4 end-to-end relative error against the
fp32 reference (tolerance 2e-2).  term2 is O(C*D) on centers/counts only,
computed host-side like the bincount; the final 128x4 per-partition partial
sums are shipped out and reduced on the host with the other cores' partials.

Per-core layout: y8 as [128, 32*128], partition p holds sorted rows
p*32..p*32+31; k-tile k = free columns [k*128,(k+1)*128) = rows {p*32+k}.
"""

import numpy as np

B = 32768
D = 128
C = 1000
NCORES = 8
BSH = B // NCORES            # 4096 rows per core
P = 128                      # SBUF partitions
RPP = BSH // P               # 32 rows per partition = # k-tiles
W = 160                      # one-hot width (max class span per shard)
CHUNK_TILES = (1, 3, 8, 20)  # escalating y DMA chunks (k-tiles each)
SQ_TILES = (12, 12, 8)       # ACT square slices (k-tiles each)

_CACHE = {}
TRACE = False                # test.py may set kernel.TRACE = True
LAST_RESULTS = None          # BassKernelResults of the last run


def _build():
    import concourse.bacc as bacc
    import concourse.mybir as mybir
    import concourse.tile as tile

    f32 = mybir.dt.float32
    f16 = mybir.dt.float16
    f8 = mybir.dt.float8e4

    nc = bacc.Bacc("TRN2", target_bir_lowering=False, debug=False,
                   enable_partition_id=False, enable_asserts=False)

    y_in = nc.dram_tensor("y8", [BSH, D], f8, kind="ExternalInput")
    lab_in = nc.dram_tensor("labf", [P, RPP], f32, kind="ExternalInput")
    ct_in = nc.dram_tensor("ctj", [P, W], f16, kind="ExternalInput")
    out = nc.dram_tensor("out", [P, 4], f32, kind="ExternalOutput")

    y_view = y_in.ap().rearrange("(p r) d -> p (r d)", p=P)

    with tile.TileContext(nc) as tc:
        with (
            tc.tile_pool(name="io", bufs=1) as io_pool,
            tc.tile_pool(name="oh", bufs=6) as oh_pool,
            tc.tile_pool(name="ps", bufs=1, space="PSUM") as psum_pool,
        ):
            # --- y fp8 over HWDGE on the sync queue, escalating chunk
            # sizes so the first k-tiles land as early as possible
            y16 = io_pool.tile([P, RPP * D], f8)
            lab_t = io_pool.tile([P, RPP], f32)
            nc.scalar.dma_start(lab_t[:], lab_in[:, :], single_packet=True)
            off = 0
            for ntile in CHUNK_TILES:
                nc.sync.dma_start(
                    y16[:, off * D:(off + ntile) * D],
                    y_view[:, off * D:(off + ntile) * D])
                off += ntile
            # centers are only needed at the very end - issue after y
            ct_t = io_pool.tile([P, W], f16)
            nc.scalar.dma_start(ct_t[:], ct_in[:, :])
            iota_t = io_pool.tile([P, W], f16)
            nc.gpsimd.iota(iota_t[:], pattern=[[1, W]], base=0,
                           channel_multiplier=0,
                           allow_small_or_imprecise_dtypes=True)

            # acc columns: 0-2 = ||y||^2 slices, 3 = v1 (cross dot)
            acc = io_pool.tile([P, 4], f32)

            # --- HAM warmup: the k-loop is PE-paced at the cold 1.2 GHz
            # stream rate; ~2.5us of dummy matmuls during the y DMA wait
            # flips the clock gate to 2.4 GHz before the real loop starts.
            warm = psum_pool.tile([P, W], f32, tag="warm")
            for _ in range(18):
                nc.tensor.matmul(warm[:], iota_t[:, 0:P], iota_t[:],
                                 start=True, stop=True)

            # --- DVE one-hots, PE accumulates S^T in PSUM.  The host
            # pairs same-label rows into slots (p, 2t)/(p, 2t+1) for
            # t < RPP//2-1, so adjacent k-tiles share one identical
            # one-hot (17 DVE ops instead of 32); the last two tiles
            # hold the unpaired leftovers with per-tile one-hots.
            sps = psum_pool.tile([P, W], f32, tag="sps")
            oh = None
            for k in range(RPP):
                if k % 2 == 0 or k >= RPP - 2:
                    oh = oh_pool.tile([P, W], f16, tag="oh")
                    nc.vector.tensor_scalar(
                        oh[:], iota_t[:], lab_t[:, k:k + 1], None,
                        mybir.AluOpType.is_equal,
                    )
                nc.tensor.matmul(
                    sps[:],
                    y16[:, k * D:(k + 1) * D],
                    oh[:],
                    start=(k == 0),
                    stop=(k == RPP - 1),
                )

            # --- term1 on ACT (fp32 internal accum); last slice is short
            # so it clears the critical tail
            soff = 0
            for j, ntile in enumerate(SQ_TILES):
                sqy = io_pool.tile([P, ntile * D], f32, tag=f"sqy{j}")
                nc.scalar.activation(
                    sqy[:], y16[:, soff * D:(soff + ntile) * D],
                    mybir.ActivationFunctionType.Square,
                    accum_out=acc[:, j:j + 1],
                )
                soff += ntile

            # --- cross partials: v1[d] = sum_c -2 * S^T[d,c] * CT[d,c]
            scr = io_pool.tile([P, W], f32)
            nc.vector.scalar_tensor_tensor(
                scr[:], sps[:], -2.0, ct_t[:],
                mybir.AluOpType.mult, mybir.AluOpType.mult,
                accum_out=acc[:, 3:4])

            # --- ship per-partition partials; host does the 128x4 sum
            nc.sync.dma_start(out[:, :], acc[:, :])

    nc.compile()
    return nc


def _get_nc():
    if "nc" not in _CACHE:
        _CACHE["nc"] = _build()
    return _CACHE["nc"]


def kernel(y, labels, centers, loss_weight):
    global LAST_RESULTS
    from concourse.bass_utils import run_bass_kernel_spmd

    y = np.asarray(y, dtype=np.float32)
    labels = np.asarray(labels).astype(np.int64)
    centers = np.ascontiguousarray(np.asarray(centers, dtype=np.float32))

    # Shard by sorted label: contiguous 4096-row slices of the sorted order.
    order = np.argsort(labels, kind="stable")
    import ml_dtypes
    ysort = y[order].astype(ml_dtypes.float8_e4m3)
    labsort = labels[order]

    # term2 = sum_c n_c ||C_c||^2 uses only labels/centers - host-side,
    # like the bincount.
    csq = np.sum(centers.astype(np.float64) ** 2, axis=1)
    nall = np.bincount(labsort, minlength=C)
    term2 = float(np.dot(nall, csq))

    nc = _get_nc()

    in_maps = []
    for j in range(NCORES):
        sl = slice(j * BSH, (j + 1) * BSH)
        lab_j = labsort[sl]
        base = int(lab_j[0])
        span = int(lab_j[-1]) - base + 1
        assert span <= W, f"class span {span} exceeds one-hot width {W}"
        # Pair consecutive same-label rows; fill slots[p, 2t]/[p, 2t+1]
        # with pair members so tiles 2t and 2t+1 share per-partition
        # labels.  Leftover pairs/singles go to the last two tiles.
        pairs = []
        singles = []
        i = 0
        while i < BSH:
            if i + 1 < BSH and lab_j[i] == lab_j[i + 1]:
                pairs.append((i, i + 1))
                i += 2
            else:
                singles.append(i)
                i += 1
        npair_slots = P * (RPP // 2 - 1)
        assert len(pairs) >= npair_slots, len(pairs)
        leftover = [r for pr in pairs[npair_slots:] for r in pr] + singles
        assert len(leftover) == 2 * P
        slots = np.empty((P, RPP), np.int64)
        for t in range(RPP // 2 - 1):
            for p in range(P):
                a, b = pairs[t * P + p]
                slots[p, 2 * t] = a
                slots[p, 2 * t + 1] = b
        lv = np.asarray(leftover).reshape(2, P).T
        slots[:, RPP - 2] = lv[:, 0]
        slots[:, RPP - 1] = lv[:, 1]
        perm = slots.reshape(-1)  # DRAM row (p*RPP+r) <- shard row perm[...]
        lab_j = lab_j[perm]
        for t in range(RPP // 2 - 1):
            assert np.array_equal(lab_j.reshape(P, RPP)[:, 2 * t],
                                  lab_j.reshape(P, RPP)[:, 2 * t + 1])
        lloc = (lab_j - base).astype(np.float32)
        # CT_j[d, c'] = centers[base + c', d], zero-padded past C
        ctj = np.zeros((P, W), np.float16)
        hi = min(base + W, C)
        ctj[:, :hi - base] = centers[base:hi].T.astype(np.float16)
        in_maps.append({
            "y8": np.ascontiguousarray(ysort[sl][perm]),
            "labf": np.ascontiguousarray(lloc.reshape(P, RPP)),
            "ctj": ctj,
        })

    res = run_bass_kernel_spmd(
        nc, in_maps, core_ids=list(range(NCORES)), trace=TRACE,
    )
    LAST_RESULTS = res

    total = sum(float(np.asarray(r["out"], np.float64).sum())
                for r in res.results)
    total += term2 + B * (C - 1) * 1e-12
    loss = total / B * float(np.asarray(loss_weight))
    return np.float32(loss)


# revision 25
# speedup vs baseline: 1.1817x; 1.1149x over previous
"""CenterLoss Trainium2 kernel (Bass/Tile, 8 NeuronCores, label-sorted shards).

loss = (sum_b clip(||y_b - centers[labels_b]||^2, 1e-12, 1e12)
        + B*(C-1)*1e-12) / B * loss_weight

The masked distmat in the reference reduces to a per-row lookup; off-label
entries of distmat*mask are exactly 0.0 and clip to 1e-12 each (closed-form
constant).  Expanding the square:

  sum_b ||y_b - c_{l_b}||^2
    = sum_b ||y_b||^2  +  sum_c n_c ||c_c||^2  -  2 sum_{c,d} S[c,d] centers[c,d]

with S = onehot^T y (class sums).

Sharding strategy (free choice per the spec): rows are sorted by label on
the host and each core takes a contiguous 4096-row slice, so each core's
shard spans only ~129 consecutive classes.  The one-hot matmul on the
TensorEngine then needs only W=160 columns instead of 1000 (~8x less PE and
DVE work).  A class straddling a core boundary contributes partial sums from
both cores; the final dot with centers is linear, so the total is exact.

The kernel is DMA/framework-floor bound (a no-op bass kernel already costs
~11.3us here; per-core DMA sustains ~230 GB/s), so y is staged in device
DRAM as fp8-e4m3 (0.5MB/core): the PE consumes it directly as the stationary
operand (mixed fp8 x fp16 matmul) and ACT squares it for term1 - no cast ops
anywhere.  fp8 staging measures ~3e-4 end-to-end relative error against the
fp32 reference (tolerance 2e-2).  term2 is O(C*D) on centers/counts only,
computed host-side like the bincount; the final 128x4 per-partition partial
sums are shipped out and reduced on the host with the other cores' partials.

Per-core layout: y8 as [128, 32*128], partition p holds sorted rows
p*32..p*32+31; k-tile k = free columns [k*128,(k+1)*128) = rows {p*32+k}.
"""

import numpy as np

B = 32768
D = 128
C = 1000
NCORES = 8
BSH = B // NCORES            # 4096 rows per core
P = 128                      # SBUF partitions
RPP = BSH // P               # 32 rows per partition = # k-tiles
W = 160                      # one-hot width (max class span per shard)
CHUNK_TILES = (1, 3, 8, 20)  # escalating y DMA chunks (k-tiles each)
SQ_TILES = (12, 12, 8)       # ACT square slices (k-tiles each)

_CACHE = {}
TRACE = False                # test.py may set kernel.TRACE = True
LAST_RESULTS = None          # BassKernelResults of the last run


def _build():
    import concourse.bacc as bacc
    import concourse.mybir as mybir
    import concourse.tile as tile

    f32 = mybir.dt.float32
    f16 = mybir.dt.float16
    f8 = mybir.dt.float8e4

    nc = bacc.Bacc("TRN2", target_bir_lowering=False, debug=False,
                   enable_partition_id=False, enable_asserts=False)

    y_in = nc.dram_tensor("y8", [BSH, D], f8, kind="ExternalInput")
    lab_in = nc.dram_tensor("labf", [P, RPP], f32, kind="ExternalInput")
    ct_in = nc.dram_tensor("ctj", [P, W], f16, kind="ExternalInput")
    out = nc.dram_tensor("out", [P, 4], f32, kind="ExternalOutput")

    y_view = y_in.ap().rearrange("(p r) d -> p (r d)", p=P)

    with tile.TileContext(nc) as tc:
        with (
            tc.tile_pool(name="io", bufs=1) as io_pool,
            tc.tile_pool(name="oh", bufs=6) as oh_pool,
            tc.tile_pool(name="ps", bufs=1, space="PSUM") as psum_pool,
        ):
            # --- y fp8 over HWDGE on the sync queue, escalating chunk
            # sizes so the first k-tiles land as early as possible
            y16 = io_pool.tile([P, RPP * D], f8)
            lab_t = io_pool.tile([P, RPP], f32)
            nc.scalar.dma_start(lab_t[:], lab_in[:, :], single_packet=True)
            off = 0
            for ntile in CHUNK_TILES:
                nc.sync.dma_start(
                    y16[:, off * D:(off + ntile) * D],
                    y_view[:, off * D:(off + ntile) * D])
                off += ntile
            # centers are only needed at the very end - issue after y
            ct_t = io_pool.tile([P, W], f16)
            nc.scalar.dma_start(ct_t[:], ct_in[:, :])
            iota_t = io_pool.tile([P, W], f16)
            nc.gpsimd.iota(iota_t[:], pattern=[[1, W]], base=0,
                           channel_multiplier=0,
                           allow_small_or_imprecise_dtypes=True)

            # acc columns: 0-2 = ||y||^2 slices, 3 = v1 (cross dot)
            acc = io_pool.tile([P, 4], f32)

            # --- HAM warmup: the k-loop is PE-paced at the cold 1.2 GHz
            # stream rate.  ~1.3us of dummy matmuls fills the PE idle
            # window before the first y chunk lands (no start delay) and
            # seeds the clock-gate activity window, so the gate flips to
            # 2.4 GHz mid-loop and the back half runs warm.
            warm = psum_pool.tile([P, W], f32, tag="warm")
            for _ in range(10):
                nc.tensor.matmul(warm[:], iota_t[:, 0:P], iota_t[:],
                                 start=True, stop=True)

            # --- DVE one-hots, PE accumulates S^T in PSUM.  The host
            # pairs same-label rows into slots (p, 2t)/(p, 2t+1) for
            # t < RPP//2-1, so adjacent k-tiles share one identical
            # one-hot (17 DVE ops instead of 32); the last two tiles
            # hold the unpaired leftovers with per-tile one-hots.
            sps = psum_pool.tile([P, W], f32, tag="sps")
            oh = None
            for k in range(RPP):
                if k % 2 == 0 or k >= RPP - 2:
                    oh = oh_pool.tile([P, W], f16, tag="oh")
                    nc.vector.tensor_scalar(
                        oh[:], iota_t[:], lab_t[:, k:k + 1], None,
                        mybir.AluOpType.is_equal,
                    )
                nc.tensor.matmul(
                    sps[:],
                    y16[:, k * D:(k + 1) * D],
                    oh[:],
                    start=(k == 0),
                    stop=(k == RPP - 1),
                )

            # --- term1 on ACT (fp32 internal accum); last slice is short
            # so it clears the critical tail
            soff = 0
            for j, ntile in enumerate(SQ_TILES):
                sqy = io_pool.tile([P, ntile * D], f32, tag=f"sqy{j}")
                nc.scalar.activation(
                    sqy[:], y16[:, soff * D:(soff + ntile) * D],
                    mybir.ActivationFunctionType.Square,
                    accum_out=acc[:, j:j + 1],
                )
                soff += ntile

            # --- cross partials: v1[d] = sum_c -2 * S^T[d,c] * CT[d,c]
            scr = io_pool.tile([P, W], f32)
            nc.vector.scalar_tensor_tensor(
                scr[:], sps[:], -2.0, ct_t[:],
                mybir.AluOpType.mult, mybir.AluOpType.mult,
                accum_out=acc[:, 3:4])

            # --- ship per-partition partials; host does the 128x4 sum
            nc.sync.dma_start(out[:, :], acc[:, :])

    nc.compile()
    return nc


def _get_nc():
    if "nc" not in _CACHE:
        _CACHE["nc"] = _build()
    return _CACHE["nc"]


def kernel(y, labels, centers, loss_weight):
    global LAST_RESULTS
    from concourse.bass_utils import run_bass_kernel_spmd

    y = np.asarray(y, dtype=np.float32)
    labels = np.asarray(labels).astype(np.int64)
    centers = np.ascontiguousarray(np.asarray(centers, dtype=np.float32))

    # Shard by sorted label: contiguous 4096-row slices of the sorted order.
    order = np.argsort(labels, kind="stable")
    import ml_dtypes
    ysort = y[order].astype(ml_dtypes.float8_e4m3)
    labsort = labels[order]

    # term2 = sum_c n_c ||C_c||^2 uses only labels/centers - host-side,
    # like the bincount.
    csq = np.sum(centers.astype(np.float64) ** 2, axis=1)
    nall = np.bincount(labsort, minlength=C)
    term2 = float(np.dot(nall, csq))

    nc = _get_nc()

    in_maps = []
    for j in range(NCORES):
        sl = slice(j * BSH, (j + 1) * BSH)
        lab_j = labsort[sl]
        base = int(lab_j[0])
        span = int(lab_j[-1]) - base + 1
        assert span <= W, f"class span {span} exceeds one-hot width {W}"
        # Pair consecutive same-label rows; fill slots[p, 2t]/[p, 2t+1]
        # with pair members so tiles 2t and 2t+1 share per-partition
        # labels.  Leftover pairs/singles go to the last two tiles.
        pairs = []
        singles = []
        i = 0
        while i < BSH:
            if i + 1 < BSH and lab_j[i] == lab_j[i + 1]:
                pairs.append((i, i + 1))
                i += 2
            else:
                singles.append(i)
                i += 1
        npair_slots = P * (RPP // 2 - 1)
        assert len(pairs) >= npair_slots, len(pairs)
        leftover = [r for pr in pairs[npair_slots:] for r in pr] + singles
        assert len(leftover) == 2 * P
        slots = np.empty((P, RPP), np.int64)
        for t in range(RPP // 2 - 1):
            for p in range(P):
                a, b = pairs[t * P + p]
                slots[p, 2 * t] = a
                slots[p, 2 * t + 1] = b
        lv = np.asarray(leftover).reshape(2, P).T
        slots[:, RPP - 2] = lv[:, 0]
        slots[:, RPP - 1] = lv[:, 1]
        perm = slots.reshape(-1)  # DRAM row (p*RPP+r) <- shard row perm[...]
        lab_j = lab_j[perm]
        for t in range(RPP // 2 - 1):
            assert np.array_equal(lab_j.reshape(P, RPP)[:, 2 * t],
                                  lab_j.reshape(P, RPP)[:, 2 * t + 1])
        lloc = (lab_j - base).astype(np.float32)
        # CT_j[d, c'] = centers[base + c', d], zero-padded past C
        ctj = np.zeros((P, W), np.float16)
        hi = min(base + W, C)
        ctj[:, :hi - base] = centers[base:hi].T.astype(np.float16)
        in_maps.append({
            "y8": np.ascontiguousarray(ysort[sl][perm]),
            "labf": np.ascontiguousarray(lloc.reshape(P, RPP)),
            "ctj": ctj,
        })

    res = run_bass_kernel_spmd(
        nc, in_maps, core_ids=list(range(NCORES)), trace=TRACE,
    )
    LAST_RESULTS = res

    total = sum(float(np.asarray(r["out"], np.float64).sum())
                for r in res.results)
    total += term2 + B * (C - 1) * 1e-12
    loss = total / B * float(np.asarray(loss_weight))
    return np.float32(loss)
